# revision 24
# baseline (speedup 1.0000x reference)
"""DeeperHNN hypergraph message passing kernel for 8 Trainium2 NeuronCores.

Strategy (sharding_hint): nodes (and incidence entries, partitioned by vertex)
are sharded across 8 cores; hyperedge aggregates are computed as per-core
partials and AllReduced (replicated, chunked for overlap); weights replicated.

v3 design vs v2 baseline (4.47ms):
  - Deferred phase-B PE tail: transposes run 2 blocks late, T-emits 4 blocks
    late, so the PE never stalls on the scalar/vector LN chain between blocks
    (v2 had a ~1.6us PE bubble per node block).
  - Gathers and one-hot S builds are prefetched several blocks ahead via
    explicit rings, keeping all 4 swdge queues fed.
  - Residual stream h lives in SBUF (no DRAM read/write per layer).
  - Per-core node re-permutation into NBV=100 balanced blocks: every phase-B
    block needs <= ~1024 gather tokens -> single gather call, 8 slots.
  - AllReduce in 8 chunks (smaller tail barrier before phase B).
  - Engine rebalance: hn-drain on DVE, rstd+reciprocal fused into one Rsqrt.

Per conv layer, per core:
  T = h @ thetaW[i] + thetaB[i]              (fp16 matmul from SBUF hT)
  Phase A: gather T rows by token -> one-hot segment matmul -> YeP
  chunked AllReduce(YeP) -> YeF (fp16, replicated)
  Phase B: gather YeF rows -> one-hot segment matmul -> relu(dv*x)
  h' = h + conv; tail: z=(h'-mu)*rinv, transpose, fused relu(g*zT+b) -> hT
"""

import numpy as np

import concourse.bacc as bacc
import concourse.bass as bass
import concourse.mybir as mybir
import concourse.tile as tile
from concourse.bass_utils import run_bass_kernel_spmd
from concourse.masks import make_identity

import ml_dtypes

P = 128
F32 = mybir.dt.float32
F16 = mybir.dt.float16
F8 = mybir.dt.float8e4
I16 = mybir.dt.int16
I32 = mybir.dt.int32
AF = mybir.ActivationFunctionType
ALU = mybir.AluOpType

F16_NP = np.float16
USE_FP8_A = False  # fp8 for the phase-A (T) gather


def _cdiv(a, b):
    return (a + b - 1) // b


def _r16(a):
    return (a + 15) // 16 * 16


# ----------------------------------------------------------------------------
# Host-side preprocessing: build per-core token tables from vidx/eidx.
# ----------------------------------------------------------------------------
class Prep:
    pass


def host_prep(vidx, eidx, N, E, C):
    """Static segment/gather structure shared by the SPMD program.

    Phase A (v->e): per core, entries sorted by eidx, grouped into NBE blocks
    of 128 edges. Per block, the token count is r16A[b] = roundup16 of the max
    per-core count; per-core tables are padded to r16A with small real indices
    and row-position -1 (one-hot never matches, so padding contributes zero).
    Phase B (e->v) is the same with (node block, eidx) swapped; nodes are
    re-permuted per core into NBV=100 balanced blocks so each block's token
    count stays near 1000 (single gather call, 8 slots).
    """
    p = Prep()
    NP = N // C
    NBE = _cdiv(_cdiv(E, P), 16) * 16  # 160 balanced edge blocks
    NBE_real = NBE                  # edges re-binned across all 160 blocks
    NBV = 100                       # balanced node blocks (12800 slots)
    NPAD = NBV * P
    EPAD = NBE * P                  # 20480
    p.N, p.E, p.C, p.NP = N, E, C, NP
    p.NBE_real, p.NBE, p.NBV, p.NPAD, p.EPAD = NBE_real, NBE, NBV, NPAD, EPAD

    vidx = np.asarray(vidx).astype(np.int64)
    eidx = np.asarray(eidx).astype(np.int64)
    de = np.bincount(eidx, minlength=E).astype(np.float64)
    dv = np.bincount(vidx, minlength=N).astype(np.float64)
    de_inv = (1.0 / np.maximum(de, 1.0)).astype(np.float32)
    dv_inv = (1.0 / np.maximum(dv, 1.0)).astype(np.float32)
    core = vidx // NP

    INF = np.iinfo(np.int64).max

    def balance(deg, nbins):
        # greedy LPT into nbins bins of capacity 128, minimizing max bin load
        n = len(deg)
        order = np.argsort(-deg, kind="stable")
        binsum = np.zeros(nbins, np.int64)
        bincnt = np.zeros(nbins, np.int64)
        pos = np.empty(n, np.int64)
        for i in order:
            b = int(np.argmin(np.where(bincnt < P, binsum, INF)))
            pos[i] = b * P + bincnt[b]
            bincnt[b] += 1
            binsum[b] += deg[i]
        return pos

    # ---- per-core balanced node permutation ----
    deg_all = np.bincount(vidx, minlength=N)
    slot_of = np.empty((C, NP), np.int64)
    for c in range(C):
        slot_of[c] = balance(deg_all[c * NP:(c + 1) * NP], NBV)
    p.slot_of = slot_of

    # ---- global balanced edge renumbering: minimize the max per-core count
    # of any block (that max is what r16A pads every core to) ----
    decv = np.zeros((E, C), np.int64)   # per-core degree of each edge
    np.add.at(decv, (eidx, core), 1)
    tot = decv.sum(1)
    order = np.argsort(-tot, kind="stable")
    binsum = np.zeros((NBE, C), np.int64)
    bincnt = np.zeros(NBE, np.int64)
    edge_slot = np.empty(E, np.int64)
    for e in order:
        cand = np.max(binsum + decv[e], axis=1)
        cand[bincnt >= P] = INF
        b = int(np.argmin(cand))
        edge_slot[e] = b * P + bincnt[b]
        bincnt[b] += 1
        binsum[b] += decv[e]
    p.edge_slot = edge_slot

    lv = vidx - core * NP
    slot = slot_of[core, lv]        # device slot of each token's node
    eslot = edge_slot[eidx]         # device slot of each token's edge

    def build_tables(key_all, val_all, nblocks):
        # key: block id = key_all // P decides the block; val: gather index
        # returns r16 (per-block padded counts), slot counts, offsets, and
        # per-core idx table + one-hot row-position table
        cnt = np.zeros((C, nblocks), np.int64)
        keys, vals = [], []
        for c in range(C):
            k = key_all[c]
            o = np.argsort(k, kind="stable")
            k = k[o]
            v = val_all[c][o]
            cnt[c] = np.bincount(k // P, minlength=nblocks)
            keys.append(k)
            vals.append(v)
        r16 = np.array([_r16(max(int(cnt[:, b].max()), 16)) for b in range(nblocks)])
        slots = (r16 + P - 1) // P
        tabOff = np.zeros(nblocks + 1, np.int64)
        np.cumsum(r16, out=tabOff[1:])
        slotOff = np.zeros(nblocks + 1, np.int64)
        np.cumsum(slots, out=slotOff[1:])
        T16 = int(tabOff[-1])
        SL = int(slotOff[-1])
        # trailing padding idxs are -1: the gather ucode trims trailing
        # negative idxs before descriptor generation (no transfer); the
        # one-hot rpos stays -1 there so stale G rows contribute zero.
        idx = np.full((C, T16), -1, np.int16)
        rpos = np.full((C, SL * P), -1.0, np.float32)
        for c in range(C):
            k, v = keys[c], vals[c]
            blk = k // P
            starts = np.searchsorted(k, np.arange(nblocks) * P)
            within = np.arange(len(k)) - starts[blk]
            idx[c, tabOff[blk] + within] = v
            rpos[c, slotOff[blk] * P + within] = k - blk * P
        return r16, slots, tabOff, slotOff, T16, SL, idx, rpos

    # ---- phase A: tokens keyed by edge slot, gather local node rows of T ----
    keyA = [eslot[core == c] for c in range(C)]
    valA = [slot[core == c] for c in range(C)]
    (p.r16A, p.slotsA, p.tabOffA, p.slotOffA, p.TA16, p.SLA,
     idxA, rposA) = build_tables(keyA, valA, NBE_real)

    # ---- phase B: tokens keyed by node slot, gather edge rows of YeF ----
    keyB = [slot[core == c] for c in range(C)]
    valB = [eslot[core == c] for c in range(C)]
    (p.r16B, p.slotsB, p.tabOffB, p.slotOffB, p.TB16, p.SLB,
     idxB, rposB) = build_tables(keyB, valB, NBV)

    p.MAXSLOT = int(max(p.slotsA.max(), p.slotsB.max()))

    # device layouts: idx wrapped into 16 partitions (replicated to 128);
    # rpos as [128, slots] columns
    def wrap_idx(idx, T16):
        return np.ascontiguousarray(
            np.tile(idx.reshape(C, T16 // 16, 16).transpose(0, 2, 1), (1, 8, 1)))

    p.idxA_w = wrap_idx(idxA, p.TA16)
    p.idxB_w = wrap_idx(idxB, p.TB16)
    p.rA_m = np.ascontiguousarray(
        rposA.reshape(C, p.SLA, P).transpose(0, 2, 1)).astype(F16_NP)
    p.rB_m = np.ascontiguousarray(
        rposB.reshape(C, p.SLB, P).transpose(0, 2, 1)).astype(F16_NP)

    # de_inv per edge-block column [128, NBE]; dv_inv per slot [C, 128, NBV]
    dec = np.zeros(EPAD, np.float32)
    dec[edge_slot] = de_inv
    p.dec = dec.reshape(NBE, P).T.copy()
    dvc = np.zeros((C, P, NBV), np.float32)
    for c in range(C):
        arr = np.zeros(NPAD, np.float32)
        arr[slot_of[c]] = dv_inv[c * NP:(c + 1) * NP]
        dvc[c] = arr.reshape(NBV, P).T
    p.dvc = dvc
    return p


# ----------------------------------------------------------------------------
# Device program
# ----------------------------------------------------------------------------
def build_program(p, IN_DIM, H, OUT, L):
    C, NP, NBV, NPAD, EPAD = p.C, p.NP, p.NBV, p.NPAD, p.EPAD
    NBE_real = p.NBE_real
    KI = IN_DIM // P  # 3
    KH = H // P       # 2
    GDTA = F8 if USE_FP8_A else F16
    NCHUNK = 4
    CHB = [0, 47, 94, 140, NBE_real]  # chunk block bounds; small last chunk
                                      # shrinks the AllReduce tail barrier

    nc = bacc.Bacc(
        "TRN2",
        target_bir_lowering=False,
        debug=False,
        enable_asserts=False,
        num_devices=C,
        num_swdge_queues=4,
    )

    # ---- I/O ----
    xT_d = nc.dram_tensor("xT", [IN_DIM, NPAD], F16, kind="ExternalInput")
    encW_d = nc.dram_tensor("encW", [IN_DIM, H], F16, kind="ExternalInput")
    encB_d = nc.dram_tensor("encB", [H], F32, kind="ExternalInput")
    thW_d = nc.dram_tensor("thW", [L, H, H], F16, kind="ExternalInput")
    thB_d = nc.dram_tensor("thB", [L, H], F16, kind="ExternalInput")
    # LN affine pre-transposed on host: [P, L*KH], column (i*KH + m) holds
    # features m*128..(m+1)*128 of layer i
    lnG_d = nc.dram_tensor("lnGT", [P, L * KH], F32, kind="ExternalInput")
    lnB_d = nc.dram_tensor("lnBT", [P, L * KH], F32, kind="ExternalInput")
    linW_d = nc.dram_tensor("linW", [H, OUT], F16, kind="ExternalInput")
    linB_d = nc.dram_tensor("linB", [OUT], F16, kind="ExternalInput")
    idxA_d = nc.dram_tensor("idxA", [P, p.TA16 // 16], I16, kind="ExternalInput")
    rA_d = nc.dram_tensor("rA", [P, p.SLA], F16, kind="ExternalInput")
    idxB_d = nc.dram_tensor("idxB", [P, p.TB16 // 16], I16, kind="ExternalInput")
    rB_d = nc.dram_tensor("rB", [P, p.SLB], F16, kind="ExternalInput")
    dv_d = nc.dram_tensor("dvc", [P, NBV], F32, kind="ExternalInput")
    dec_d = nc.dram_tensor("dec", [P, p.NBE], F32, kind="ExternalInput")
    out_d = nc.dram_tensor("out", [NPAD, OUT], F32, kind="ExternalOutput")

    # ---- internals ----
    T_d = nc.dram_tensor("T_t", [NPAD, H], GDTA)
    YePc_d = [nc.dram_tensor(f"YeP{k}", [(CHB[k + 1] - CHB[k]) * P, H], F16)
              for k in range(NCHUNK)]
    YeF_d = nc.dram_tensor("YeF", [EPAD, H], F16, addr_space="Shared")

    SAMAX = int(p.slotsA.max())
    SBMAX = int(p.slotsB.max())
    MS = p.MAXSLOT
    GPA_BUFS, PREGA, PRESA = 10, 9, 3
    GPB_BUFS, PREGB, PRESB = 10, 9, 3
    ZR = 6
    HTR = 8                  # hT ring depth (blocks); 512-col encoder chunks
                             # span 4 slots, so 8 never wraps mid-chunk

    from contextlib import ExitStack
    with tile.TileContext(nc) as tc, ExitStack() as es:
        const = es.enter_context(tc.tile_pool(name="const", bufs=1))
        meta = es.enter_context(tc.tile_pool(name="meta", bufs=1))
        gpa = es.enter_context(tc.tile_pool(name="gpa", bufs=GPA_BUFS))
        gpb = es.enter_context(tc.tile_pool(name="gpb", bufs=GPB_BUFS))
        spool = es.enter_context(tc.tile_pool(name="spool", bufs=4))
        wrk = es.enter_context(tc.tile_pool(name="wrk", bufs=2))
        stat = es.enter_context(tc.tile_pool(name="stat", bufs=4))
        opool = es.enter_context(tc.tile_pool(name="opool", bufs=3))
        psA = es.enter_context(tc.tile_pool(name="psA", bufs=3, space="PSUM"))
        psT = es.enter_context(tc.tile_pool(name="psT", bufs=2, space="PSUM"))
        psE = es.enter_context(tc.tile_pool(name="psE", bufs=3, space="PSUM"))

        # ---- constants ----
        iota_i = const.tile([P, MS, P], I32)
        nc.gpsimd.iota(iota_i[:, :, :], pattern=[[0, MS], [1, P]], base=0,
                       channel_multiplier=0)
        iota_f = const.tile([P, MS, P], F16)
        nc.vector.tensor_copy(iota_f[:, :, :], iota_i[:, :, :])
        ident = const.tile([P, P], F16)
        make_identity(nc, ident[:, :])
        ones1 = const.tile([1, P], F16)
        nc.vector.memset(ones1[:, :], 1.0)
        epsc = const.tile([P, 1], F32)
        nc.vector.memset(epsc[:, :], 1e-5)

        # transposed activations hT as a short ring (written by the LN tail /
        # encoder, read by the T-emit a couple of blocks later) and the
        # SBUF-resident residual stream h [node block, feat]
        hT_sb = const.tile([P, KH, HTR * P], F16)
        h_sb = const.tile([P, NBV, H], F16)

        # z ring (LN-normalized activations, consumed 2 blocks later by PE)
        z_ring = [const.tile([P, H], F16, tag=f"zr{i}", name=f"zr{i}")
                  for i in range(ZR)]

        # zero ALL G pool buffers once so padding rows are finite
        # (uninitialized SBUF can hold NaN bit patterns; 0 * NaN = NaN)
        for b in range(GPA_BUFS):
            ga = gpa.tile([P, SAMAX, H], GDTA, tag="GA")
            nc.vector.memset(ga[:, :, :], 0.0)
        for b in range(GPB_BUFS):
            gb = gpb.tile([P, SBMAX, H], F16, tag="GB")
            nc.vector.memset(gb[:, :, :], 0.0)

        # weights
        encW_t = []
        for k in range(KI):
            row = []
            for m in range(KH):
                t = const.tile([P, P], F16, tag=f"encW{k}{m}")
                nc.sync.dma_start(t[:, :], encW_d[k * P:(k + 1) * P, m * P:(m + 1) * P])
                row.append(t)
            encW_t.append(row)
        encB_c = []
        for m in range(KH):
            t = const.tile([P, 1], F32, tag=f"encB{m}")
            nc.sync.dma_start(t[:, :], encB_d[m * P:(m + 1) * P, None])
            encB_c.append(t)
        thW_t = []
        for i in range(L):
            row = []
            for k in range(KH):
                t = const.tile([P, H], F16, tag=f"thW{i}{k}")
                nc.sync.dma_start(t[:, :], thW_d[i, k * P:(k + 1) * P, :])
                row.append(t)
            thW_t.append(row)
        thB_t = []
        for i in range(L):
            t = const.tile([1, H], F16, tag=f"thB{i}")
            nc.sync.dma_start(t[:, :], thB_d[i:i + 1, :])
            thB_t.append(t)
        linW_t = []
        for k in range(KH):
            t = const.tile([P, OUT], F16, tag=f"linW{k}")
            nc.sync.dma_start(t[:, :], linW_d[k * P:(k + 1) * P, :])
            linW_t.append(t)
        linB_t = const.tile([1, OUT], F16)
        nc.sync.dma_start(linB_t[:, :], linB_d[None, :])
        # LN affine in transposed space: per-feature -> per-partition columns
        lnG_t, lnB_t = [], []
        for i in range(L):
            g = const.tile([P, KH], F32, tag=f"lnG{i}")
            b = const.tile([P, KH], F32, tag=f"lnB{i}")
            nc.sync.dma_start(g[:, :], lnG_d[:, i * KH:(i + 1) * KH])
            nc.sync.dma_start(b[:, :], lnB_d[:, i * KH:(i + 1) * KH])
            lnG_t.append(g)
            lnB_t.append(b)

        # metadata
        idxA_t = meta.tile([P, p.TA16 // 16], I16)
        nc.sync.dma_start(idxA_t[:, :], idxA_d[:, :])
        rA_t = meta.tile([P, p.SLA], F16)
        nc.sync.dma_start(rA_t[:, :], rA_d[:, :])
        dec_t = meta.tile([P, p.NBE], F32)
        nc.sync.dma_start(dec_t[:, :], dec_d[:, :])
        idxB_t = meta.tile([P, p.TB16 // 16], I16)
        nc.sync.dma_start(idxB_t[:, :], idxB_d[:, :])
        rB_t = meta.tile([P, p.SLB], F16)
        nc.sync.dma_start(rB_t[:, :], rB_d[:, :])
        dv_t = meta.tile([P, NBV], F32)
        nc.sync.dma_start(dv_t[:, :], dv_d[:, :])

        CW = 512
        qn_state = [0]

        # r16A/r16B are uniform after host-side balancing: hoist the
        # num_idxs_reg loads so each gather doesn't emit its own MOVE on the
        # Pool stream (dispatch overhead per call)
        uniA = len(set(p.r16A.tolist())) == 1
        uniB = len(set(p.r16B.tolist())) == 1
        regA = nc.gpsimd.to_reg(int(p.r16A[0])) if uniA else None
        regB = nc.gpsimd.to_reg(int(p.r16B[0])) if uniB else None


        def next_q():
            q = qn_state[0]
            qn_state[0] = (q + 1) % 4
            return q

        def emit_T(li, rb):
            # T[rb] = h @ thetaW[li] + thetaB[li], written fp16 to T_d
            hc = (rb % HTR) * P
            psw = psE.tile([P, CW], F32, tag="psE")
            for k in range(KH):
                nc.tensor.matmul(psw[:, :H], lhsT=hT_sb[:, k, hc:hc + P],
                                 rhs=thW_t[li][k][:, :],
                                 start=(k == 0), stop=False)
            nc.tensor.matmul(psw[:, :H], lhsT=ones1[:1, :], rhs=thB_t[li][:1, :],
                             start=False, stop=True)
            Tb = opool.tile([P, H], GDTA, tag="Tout")
            nc.scalar.activation(Tb[:, :], psw[:, :H], AF.Copy)
            nc.sync.dma_start(T_d[rb * P:rb * P + P, :], Tb[:, :])

        def emit_final(rb):
            # out[rb] = t @ linW + linB (t = relu(LN_0(h)) already in hT)
            hc = (rb % HTR) * P
            psw = psE.tile([P, CW], F32, tag="psE")
            for k in range(KH):
                nc.tensor.matmul(psw[:, :OUT], lhsT=hT_sb[:, k, hc:hc + P],
                                 rhs=linW_t[k][:, :], start=(k == 0), stop=False)
            nc.tensor.matmul(psw[:, :OUT], lhsT=ones1[:1, :], rhs=linB_t[:1, :],
                             start=False, stop=True)
            ob = opool.tile([P, OUT], F32, tag="finout")
            nc.scalar.activation(ob[:, :], psw[:, :OUT], AF.Copy)
            nc.sync.dma_start(out_d[rb * P:rb * P + P, :], ob[:, :])

        # ------------------------------------------------------------------
        # Encoder: hT[:, m, :] = (x @ encW + encB)^T, feature-major directly.
        # Layer 0's T-matmul is fused in per 512-column chunk.
        # ------------------------------------------------------------------
        for c0 in range(0, NPAD, CW):
            ncols = min(CW, NPAD - c0)
            hc = c0 % (HTR * P)
            xc = wrk.tile([P, KI, CW], F16, tag="xc")
            nc.sync.dma_start(
                xc[:, :, :ncols],
                xT_d.ap().rearrange("(k q) n -> q k n", q=P)[:, :, c0:c0 + ncols],
            )
            for m in range(KH):
                ps = psE.tile([P, CW], F32, tag="psE")
                for k in range(KI):
                    nc.tensor.matmul(ps[:, :ncols], lhsT=encW_t[k][m][:, :],
                                     rhs=xc[:, k, :ncols],
                                     start=(k == 0), stop=(k == KI - 1))
                nc.scalar.activation(hT_sb[:, m, hc:hc + ncols], ps[:, :ncols],
                                     AF.Identity, bias=encB_c[m][:, :], scale=1.0)
            for rb in range(c0 // P, (c0 + ncols) // P):
                emit_T(0, rb)

        # ------------------------------------------------------------------
        # Conv layers
        # ------------------------------------------------------------------
        def gatherA(eb):
            r16 = int(p.r16A[eb])
            t0 = int(p.tabOffA[eb])
            sb = int(p.slotsA[eb])
            G = gpa.tile([P, SAMAX, H], GDTA, tag="GA")
            nc.gpsimd.dma_gather(
                out_ap=G[:, :sb, :],
                in_ap=T_d[:, :],
                idxs_ap=idxA_t[:, t0 // 16:(t0 + r16) // 16],
                num_idxs=r16,
                num_idxs_reg=regA if uniA else r16,
                elem_size=H,
                queue_num=next_q(),
            )
            return G

        def buildSA(eb):
            sb = int(p.slotsA[eb])
            s0 = int(p.slotOffA[eb])
            S = spool.tile([P, MS, P], GDTA, tag="SA")
            rb_ap = rA_t[:, s0:s0 + sb].unsqueeze(2).broadcast_to([P, sb, P])
            nc.vector.tensor_tensor(S[:, :sb, :], iota_f[:, :sb, :], rb_ap,
                                    op=ALU.is_equal)
            return S

        def gatherB(vb):
            r16 = int(p.r16B[vb])
            t0 = int(p.tabOffB[vb])
            G = gpb.tile([P, SBMAX, H], F16, tag="GB")
            g0 = 0
            while g0 < r16:
                gn = min(1024, r16 - g0)
                nc.gpsimd.dma_gather(
                    out_ap=G[:, g0 // P:g0 // P + _cdiv(gn, P), :],
                    in_ap=YeF_d[:, :],
                    idxs_ap=idxB_t[:, (t0 + g0) // 16:(t0 + g0 + gn) // 16],
                    num_idxs=gn,
                    num_idxs_reg=(regB if (uniB and gn == int(p.r16B[0])) else gn),
                    elem_size=H,
                    queue_num=next_q(),
                )
                g0 += gn
            return G

        def buildSB(vb):
            sb = int(p.slotsB[vb])
            s0 = int(p.slotOffB[vb])
            S = spool.tile([P, MS, P], F16, tag="SB")
            rb_ap = rB_t[:, s0:s0 + sb].unsqueeze(2).broadcast_to([P, sb, P])
            nc.vector.tensor_tensor(S[:, :sb, :], iota_f[:, :sb, :], rb_ap,
                                    op=ALU.is_equal)
            return S

        for li in range(L):
            # ---- Phase A: partial Ye, chunked AllReduce ----
            Gq = {}
            Sq = {}
            for e in range(min(PREGA, NBE_real)):
                Gq[e] = gatherA(e)
            for e in range(min(PRESA, NBE_real)):
                Sq[e] = buildSA(e)
            for eb in range(NBE_real):
                if eb + PREGA < NBE_real:
                    Gq[eb + PREGA] = gatherA(eb + PREGA)
                if eb + PRESA < NBE_real:
                    Sq[eb + PRESA] = buildSA(eb + PRESA)
                G = Gq.pop(eb)
                S = Sq.pop(eb)
                sb = int(p.slotsA[eb])
                ps = psA.tile([P, H], F32, tag="ps256")
                for s in range(sb):
                    nc.tensor.matmul(ps[:, :], lhsT=S[:, s, :], rhs=G[:, s, :],
                                     start=(s == 0), stop=(s == sb - 1))
                yeb = opool.tile([P, H], F16, tag="yeg", bufs=4)
                ck = next(k for k in range(NCHUNK) if CHB[k] <= eb < CHB[k + 1])
                er = (eb - CHB[ck]) * P
                nc.scalar.activation(yeb[:, :], ps[:, :], AF.Copy,
                                     scale=dec_t[:, eb:eb + 1])
                nc.sync.dma_start(YePc_d[ck][er:er + P, :], yeb[:, :])

                # AllReduce each chunk 12 blocks after its last block so the
                # Pool stream never stalls on the chunk's YeP write sems
                # (a stalled Pool head starves gather dispatch for ~24us)
                if eb >= 12 and (eb - 12 + 1) in CHB[1:NCHUNK]:
                    ck2 = CHB.index(eb - 12 + 1) - 1
                    nc.gpsimd.collective_compute(
                        "AllReduce", ALU.add,
                        replica_groups=[list(range(C))],
                        ins=[YePc_d[ck2].ap()[:, :]],
                        outs=[YeF_d.ap()[CHB[ck2] * P:CHB[ck2 + 1] * P, :]],
                    )
            nc.gpsimd.collective_compute(
                "AllReduce", ALU.add,
                replica_groups=[list(range(C))],
                ins=[YePc_d[NCHUNK - 1].ap()[:, :]],
                outs=[YeF_d.ap()[CHB[NCHUNK - 1] * P:, :]],
            )

            # ---- Phase B: conv + residual + LN tail (PE tail deferred) ----
            lnxt = li + 1 if li + 1 < L else 0

            def tail_transpose(vt):
                z = z_ring[vt % ZR]
                hc = (vt % HTR) * P
                for m in range(KH):
                    pst = psT.tile([P, P], F16, tag="psT")
                    nc.tensor.transpose(pst[:, :], z[:, m * P:(m + 1) * P],
                                        ident[:, :])
                    nc.scalar.activation(
                        hT_sb[:, m, hc:hc + P], pst[:, :], AF.Relu,
                        bias=lnB_t[lnxt][:, m:m + 1], scale=lnG_t[lnxt][:, m:m + 1])

            def emit_at(vt):
                if li + 1 < L:
                    emit_T(li + 1, vt)
                else:
                    emit_final(vt)

            Gq = {}
            Sq = {}
            for v in range(min(PREGB, NBV)):
                Gq[v] = gatherB(v)
            for v in range(min(PRESB, NBV)):
                Sq[v] = buildSB(v)
            for vb in range(NBV):
                if vb + PREGB < NBV:
                    Gq[vb + PREGB] = gatherB(vb + PREGB)
                if vb + PRESB < NBV:
                    Sq[vb + PRESB] = buildSB(vb + PRESB)
                G = Gq.pop(vb)
                S = Sq.pop(vb)
                sb = int(p.slotsB[vb])
                ps = psA.tile([P, H], F32, tag="ps256")
                for s in range(sb):
                    nc.tensor.matmul(ps[:, :], lhsT=S[:, s, :], rhs=G[:, s, :],
                                     start=(s == 0), stop=(s == sb - 1))
                # hn = relu(dv * x) (== dv * relu(x), dv >= 0), on DVE
                hslice = h_sb[:, vb, :]
                if li == 0:
                    nc.vector.tensor_scalar(hslice, ps[:, :], dv_t[:, vb:vb + 1],
                                            0.0, op0=ALU.mult, op1=ALU.max)
                else:
                    hn = wrk.tile([P, H], F16, tag="hn")
                    nc.vector.tensor_scalar(hn[:, :], ps[:, :], dv_t[:, vb:vb + 1],
                                            0.0, op0=ALU.mult, op1=ALU.max)
                    nc.vector.tensor_add(hslice, hslice, hn[:, :])
                # tail: z = (h - mu) * rinv (transpose + affine deferred)
                st6 = stat.tile([P, 6], F32, tag="st6")
                nc.vector.bn_stats(st6[:, :], hslice)
                mv = stat.tile([P, 2], F32, tag="mv")
                nc.vector.bn_aggr(mv[:, :], st6[:, :])
                rstd = stat.tile([P, 1], F32, tag="rstd")
                nc.scalar.activation(rstd[:, :], mv[:, 1:2], AF.Sqrt,
                                     bias=epsc[:, :], scale=1.0)
                rinv = stat.tile([P, 1], F32, tag="rinv")
                nc.vector.reciprocal(rinv[:, :], rstd[:, :])
                nmr = stat.tile([P, 1], F32, tag="nmr")
                nc.vector.tensor_scalar(nmr[:, :], mv[:, 0:1], rinv[:, :], -1.0,
                                        op0=ALU.mult, op1=ALU.mult)
                z = z_ring[vb % ZR]
                nc.scalar.activation(z[:, :], hslice, AF.Identity,
                                     bias=nmr[:, :], scale=rinv[:, :])
                if vb >= 2:
                    tail_transpose(vb - 2)
                if vb >= 4:
                    emit_at(vb - 4)
            tail_transpose(NBV - 2)
            tail_transpose(NBV - 1)
            for vt in (NBV - 4, NBV - 3, NBV - 2, NBV - 1):
                emit_at(vt)

    nc.compile()
    return nc


# ----------------------------------------------------------------------------
# Full pipeline: prep + build + run
# ----------------------------------------------------------------------------
def run_full(x, vidx, eidx, encW, encB, thetaW, thetaB, lnG, lnB, linW, linB,
             N, E, C, trace=False, nc_cache=None, **runkw):
    IN_DIM = x.shape[1]
    H = encW.shape[1]
    OUT = linW.shape[1]
    L = thetaW.shape[0]

    p = host_prep(np.asarray(vidx), np.asarray(eidx), N, E, C)
    nc = nc_cache if nc_cache is not None else build_program(p, IN_DIM, H, OUT, L)

    x = np.asarray(x, np.float32)
    NP, NPAD = p.NP, p.NPAD
    in_maps = []
    for c in range(C):
        xs = x[c * NP:(c + 1) * NP]
        xT = np.zeros((IN_DIM, NPAD), F16_NP)
        xT[:, p.slot_of[c]] = xs.T.astype(F16_NP)
        in_maps.append(dict(
            xT=xT,
            encW=np.asarray(encW, F16_NP),
            encB=np.asarray(encB, np.float32),
            thW=np.asarray(thetaW, F16_NP),
            thB=np.asarray(thetaB, F16_NP),
            lnGT=np.ascontiguousarray(
                np.asarray(lnG, np.float32).reshape(4, 2, P).transpose(2, 0, 1).reshape(P, 8)),
            lnBT=np.ascontiguousarray(
                np.asarray(lnB, np.float32).reshape(4, 2, P).transpose(2, 0, 1).reshape(P, 8)),
            linW=np.asarray(linW, F16_NP),
            linB=np.asarray(linB, F16_NP),
            idxA=p.idxA_w[c],
            rA=p.rA_m[c],
            idxB=p.idxB_w[c],
            rB=p.rB_m[c],
            dvc=p.dvc[c],
            dec=p.dec,
        ))

    res = run_bass_kernel_spmd(nc, in_maps, core_ids=list(range(C)), trace=trace, **runkw)
    out = np.concatenate(
        [res.results[c]["out"][p.slot_of[c]] for c in range(C)], axis=0)
    return out, res, nc, p


# hardcoded problem configuration (nn_DeeperHNN_88295937671288)
_N, _E, _NNZ = 100000, 20000, 800000
_C = 8

_nc_cache = None


def kernel(x, vidx, eidx, encW, encB, thetaW, thetaB, lnG, lnB, linW, linB):
    global _nc_cache
    out, res, nc, p = run_full(
        x, vidx, eidx, encW, encB, thetaW, thetaB, lnG, lnB, linW, linB,
        N=_N, E=_E, C=_C, nc_cache=None,
    )
    _nc_cache = nc
    return out.astype(np.float32)


# revision 26
# speedup vs baseline: 1.1212x; 1.1212x over previous
"""DeeperHNN hypergraph message passing kernel for 8 Trainium2 NeuronCores.

Strategy (sharding_hint): nodes (and incidence entries, partitioned by vertex)
are sharded across 8 cores; hyperedge aggregates are computed as per-core
partials and AllReduced (replicated, chunked for overlap); weights replicated.

v3 design vs v2 baseline (4.47ms):
  - Deferred phase-B PE tail: transposes run 2 blocks late, T-emits 4 blocks
    late, so the PE never stalls on the scalar/vector LN chain between blocks
    (v2 had a ~1.6us PE bubble per node block).
  - Gathers and one-hot S builds are prefetched several blocks ahead via
    explicit rings, keeping all 4 swdge queues fed.
  - Residual stream h lives in SBUF (no DRAM read/write per layer).
  - Per-core node re-permutation into NBV=100 balanced blocks: every phase-B
    block needs <= ~1024 gather tokens -> single gather call, 8 slots.
  - AllReduce in 8 chunks (smaller tail barrier before phase B).
  - Engine rebalance: hn-drain on DVE, rstd+reciprocal fused into one Rsqrt.

Per conv layer, per core:
  T = h @ thetaW[i] + thetaB[i]              (fp16 matmul from SBUF hT)
  Phase A: gather T rows by token -> one-hot segment matmul -> YeP
  chunked AllReduce(YeP) -> YeF (fp16, replicated)
  Phase B: gather YeF rows -> one-hot segment matmul -> relu(dv*x)
  h' = h + conv; tail: z=(h'-mu)*rinv, transpose, fused relu(g*zT+b) -> hT
"""

import numpy as np

import concourse.bacc as bacc
import concourse.bass as bass
import concourse.mybir as mybir
import concourse.tile as tile
from concourse.bass_utils import run_bass_kernel_spmd
from concourse.masks import make_identity

import ml_dtypes

P = 128
F32 = mybir.dt.float32
F16 = mybir.dt.float16
F8 = mybir.dt.float8e4
I16 = mybir.dt.int16
I32 = mybir.dt.int32
AF = mybir.ActivationFunctionType
ALU = mybir.AluOpType

F16_NP = np.float16
USE_FP8_A = True   # fp8 for the phase-A (T) gather


def _cdiv(a, b):
    return (a + b - 1) // b


def _r16(a):
    return (a + 15) // 16 * 16


# ----------------------------------------------------------------------------
# Host-side preprocessing: build per-core token tables from vidx/eidx.
# ----------------------------------------------------------------------------
class Prep:
    pass


def host_prep(vidx, eidx, N, E, C):
    """Static segment/gather structure shared by the SPMD program.

    Phase A (v->e): per core, entries sorted by eidx, grouped into NBE blocks
    of 128 edges. Per block, the token count is r16A[b] = roundup16 of the max
    per-core count; per-core tables are padded to r16A with small real indices
    and row-position -1 (one-hot never matches, so padding contributes zero).
    Phase B (e->v) is the same with (node block, eidx) swapped; nodes are
    re-permuted per core into NBV=100 balanced blocks so each block's token
    count stays near 1000 (single gather call, 8 slots).
    """
    p = Prep()
    NP = N // C
    NBE = _cdiv(_cdiv(E, P), 16) * 16  # 160 balanced edge blocks
    NBE_real = NBE                  # edges re-binned across all 160 blocks
    NBV = 100                       # balanced node blocks (12800 slots)
    NPAD = NBV * P
    EPAD = NBE * P                  # 20480
    p.N, p.E, p.C, p.NP = N, E, C, NP
    p.NBE_real, p.NBE, p.NBV, p.NPAD, p.EPAD = NBE_real, NBE, NBV, NPAD, EPAD

    vidx = np.asarray(vidx).astype(np.int64)
    eidx = np.asarray(eidx).astype(np.int64)
    de = np.bincount(eidx, minlength=E).astype(np.float64)
    dv = np.bincount(vidx, minlength=N).astype(np.float64)
    de_inv = (1.0 / np.maximum(de, 1.0)).astype(np.float32)
    dv_inv = (1.0 / np.maximum(dv, 1.0)).astype(np.float32)
    core = vidx // NP

    INF = np.iinfo(np.int64).max

    def balance(deg, nbins):
        # greedy LPT into nbins bins of capacity 128, minimizing max bin load
        n = len(deg)
        order = np.argsort(-deg, kind="stable")
        binsum = np.zeros(nbins, np.int64)
        bincnt = np.zeros(nbins, np.int64)
        pos = np.empty(n, np.int64)
        for i in order:
            b = int(np.argmin(np.where(bincnt < P, binsum, INF)))
            pos[i] = b * P + bincnt[b]
            bincnt[b] += 1
            binsum[b] += deg[i]
        return pos

    # ---- per-core balanced node permutation ----
    deg_all = np.bincount(vidx, minlength=N)
    slot_of = np.empty((C, NP), np.int64)
    for c in range(C):
        slot_of[c] = balance(deg_all[c * NP:(c + 1) * NP], NBV)
    p.slot_of = slot_of

    # ---- global balanced edge renumbering: minimize the max per-core count
    # of any block (that max is what r16A pads every core to) ----
    decv = np.zeros((E, C), np.int64)   # per-core degree of each edge
    np.add.at(decv, (eidx, core), 1)
    tot = decv.sum(1)
    order = np.argsort(-tot, kind="stable")
    binsum = np.zeros((NBE, C), np.int64)
    bincnt = np.zeros(NBE, np.int64)
    edge_slot = np.empty(E, np.int64)
    for e in order:
        cand = np.max(binsum + decv[e], axis=1)
        cand[bincnt >= P] = INF
        b = int(np.argmin(cand))
        edge_slot[e] = b * P + bincnt[b]
        bincnt[b] += 1
        binsum[b] += decv[e]
    p.edge_slot = edge_slot

    lv = vidx - core * NP
    slot = slot_of[core, lv]        # device slot of each token's node
    eslot = edge_slot[eidx]         # device slot of each token's edge

    def build_tables(key_all, val_all, nblocks):
        # key: block id = key_all // P decides the block; val: gather index
        # returns r16 (per-block padded counts), slot counts, offsets, and
        # per-core idx table + one-hot row-position table
        cnt = np.zeros((C, nblocks), np.int64)
        keys, vals = [], []
        for c in range(C):
            k = key_all[c]
            o = np.argsort(k, kind="stable")
            k = k[o]
            v = val_all[c][o]
            cnt[c] = np.bincount(k // P, minlength=nblocks)
            keys.append(k)
            vals.append(v)
        r16 = np.array([_r16(max(int(cnt[:, b].max()), 16)) for b in range(nblocks)])
        slots = (r16 + P - 1) // P
        tabOff = np.zeros(nblocks + 1, np.int64)
        np.cumsum(r16, out=tabOff[1:])
        slotOff = np.zeros(nblocks + 1, np.int64)
        np.cumsum(slots, out=slotOff[1:])
        T16 = int(tabOff[-1])
        SL = int(slotOff[-1])
        # trailing padding idxs are -1: the gather ucode trims trailing
        # negative idxs before descriptor generation (no transfer); the
        # one-hot rpos stays -1 there so stale G rows contribute zero.
        idx = np.full((C, T16), -1, np.int16)
        rpos = np.full((C, SL * P), -1.0, np.float32)
        for c in range(C):
            k, v = keys[c], vals[c]
            blk = k // P
            starts = np.searchsorted(k, np.arange(nblocks) * P)
            within = np.arange(len(k)) - starts[blk]
            idx[c, tabOff[blk] + within] = v
            rpos[c, slotOff[blk] * P + within] = k - blk * P
        return r16, slots, tabOff, slotOff, T16, SL, idx, rpos

    # ---- phase A: tokens keyed by edge slot, gather local node rows of T ----
    keyA = [eslot[core == c] for c in range(C)]
    valA = [slot[core == c] for c in range(C)]
    (p.r16A, p.slotsA, p.tabOffA, p.slotOffA, p.TA16, p.SLA,
     idxA, rposA) = build_tables(keyA, valA, NBE_real)

    # ---- phase B: tokens keyed by node slot, gather edge rows of YeF ----
    keyB = [slot[core == c] for c in range(C)]
    valB = [eslot[core == c] for c in range(C)]
    (p.r16B, p.slotsB, p.tabOffB, p.slotOffB, p.TB16, p.SLB,
     idxB, rposB) = build_tables(keyB, valB, NBV)

    p.MAXSLOT = int(max(p.slotsA.max(), p.slotsB.max()))

    # device layouts: idx wrapped into 16 partitions (replicated to 128);
    # rpos as [128, slots] columns
    def wrap_idx(idx, T16):
        return np.ascontiguousarray(
            np.tile(idx.reshape(C, T16 // 16, 16).transpose(0, 2, 1), (1, 8, 1)))

    p.idxA_w = wrap_idx(idxA, p.TA16)
    p.idxB_w = wrap_idx(idxB, p.TB16)
    p.rA_m = np.ascontiguousarray(
        rposA.reshape(C, p.SLA, P).transpose(0, 2, 1)).astype(F16_NP)
    p.rB_m = np.ascontiguousarray(
        rposB.reshape(C, p.SLB, P).transpose(0, 2, 1)).astype(F16_NP)

    # de_inv per edge-block column [128, NBE]; dv_inv per slot [C, 128, NBV]
    dec = np.zeros(EPAD, np.float32)
    dec[edge_slot] = de_inv
    p.dec = dec.reshape(NBE, P).T.copy()
    dvc = np.zeros((C, P, NBV), np.float32)
    for c in range(C):
        arr = np.zeros(NPAD, np.float32)
        arr[slot_of[c]] = dv_inv[c * NP:(c + 1) * NP]
        dvc[c] = arr.reshape(NBV, P).T
    p.dvc = dvc
    return p


# ----------------------------------------------------------------------------
# Device program
# ----------------------------------------------------------------------------
def build_program(p, IN_DIM, H, OUT, L):
    C, NP, NBV, NPAD, EPAD = p.C, p.NP, p.NBV, p.NPAD, p.EPAD
    NBE_real = p.NBE_real
    KI = IN_DIM // P  # 3
    KH = H // P       # 2
    GDTA = F8 if USE_FP8_A else F16
    NCHUNK = 4
    CHB = [0, 47, 94, 140, NBE_real]  # chunk block bounds; small last chunk
                                      # shrinks the AllReduce tail barrier

    nc = bacc.Bacc(
        "TRN2",
        target_bir_lowering=False,
        debug=False,
        enable_asserts=False,
        num_devices=C,
        num_swdge_queues=4,
    )

    # ---- I/O ----
    xT_d = nc.dram_tensor("xT", [IN_DIM, NPAD], F16, kind="ExternalInput")
    encW_d = nc.dram_tensor("encW", [IN_DIM, H], F16, kind="ExternalInput")
    encB_d = nc.dram_tensor("encB", [H], F32, kind="ExternalInput")
    thW_d = nc.dram_tensor("thW", [L, H, H], F16, kind="ExternalInput")
    thB_d = nc.dram_tensor("thB", [L, H], F16, kind="ExternalInput")
    # LN affine pre-transposed on host: [P, L*KH], column (i*KH + m) holds
    # features m*128..(m+1)*128 of layer i
    lnG_d = nc.dram_tensor("lnGT", [P, L * KH], F32, kind="ExternalInput")
    lnB_d = nc.dram_tensor("lnBT", [P, L * KH], F32, kind="ExternalInput")
    linW_d = nc.dram_tensor("linW", [H, OUT], F16, kind="ExternalInput")
    linB_d = nc.dram_tensor("linB", [OUT], F16, kind="ExternalInput")
    idxA_d = nc.dram_tensor("idxA", [P, p.TA16 // 16], I16, kind="ExternalInput")
    rA_d = nc.dram_tensor("rA", [P, p.SLA], F16, kind="ExternalInput")
    idxB_d = nc.dram_tensor("idxB", [P, p.TB16 // 16], I16, kind="ExternalInput")
    rB_d = nc.dram_tensor("rB", [P, p.SLB], F16, kind="ExternalInput")
    dv_d = nc.dram_tensor("dvc", [P, NBV], F32, kind="ExternalInput")
    dec_d = nc.dram_tensor("dec", [P, p.NBE], F32, kind="ExternalInput")
    out_d = nc.dram_tensor("out", [NPAD, OUT], F32, kind="ExternalOutput")

    # ---- internals ----
    T_d = nc.dram_tensor("T_t", [NPAD, H], GDTA)
    YePc_d = [nc.dram_tensor(f"YeP{k}", [(CHB[k + 1] - CHB[k]) * P, H], F16)
              for k in range(NCHUNK)]
    YeF_d = nc.dram_tensor("YeF", [EPAD, H], F16, addr_space="Shared")

    SAMAX = int(p.slotsA.max())
    SBMAX = int(p.slotsB.max())
    MS = p.MAXSLOT
    GPA_BUFS, PREGA, PRESA = 10, 8, 2
    GPB_BUFS, PREGB, PRESB = 10, 8, 2
    ZR = 4
    HTR = 8                  # hT ring depth (blocks); 512-col encoder chunks
                             # span 4 slots, so 8 never wraps mid-chunk

    from contextlib import ExitStack
    with tile.TileContext(nc) as tc, ExitStack() as es:
        const = es.enter_context(tc.tile_pool(name="const", bufs=1))
        meta = es.enter_context(tc.tile_pool(name="meta", bufs=1))
        gpa = es.enter_context(tc.tile_pool(name="gpa", bufs=GPA_BUFS))
        gpb = es.enter_context(tc.tile_pool(name="gpb", bufs=GPB_BUFS))
        spool = es.enter_context(tc.tile_pool(name="spool", bufs=3))
        wrk = es.enter_context(tc.tile_pool(name="wrk", bufs=2))
        stat = es.enter_context(tc.tile_pool(name="stat", bufs=4))
        opool = es.enter_context(tc.tile_pool(name="opool", bufs=3))
        psA = es.enter_context(tc.tile_pool(name="psA", bufs=3, space="PSUM"))
        psT = es.enter_context(tc.tile_pool(name="psT", bufs=2, space="PSUM"))
        psE = es.enter_context(tc.tile_pool(name="psE", bufs=3, space="PSUM"))

        # ---- constants ----
        iota_i = const.tile([P, MS, P], I32)
        nc.gpsimd.iota(iota_i[:, :, :], pattern=[[0, MS], [1, P]], base=0,
                       channel_multiplier=0)
        iota_f = const.tile([P, MS, P], F16)
        nc.vector.tensor_copy(iota_f[:, :, :], iota_i[:, :, :])
        ident = const.tile([P, P], F16)
        make_identity(nc, ident[:, :])
        ones1 = const.tile([1, P], F16)
        nc.vector.memset(ones1[:, :], 1.0)
        epsc = const.tile([P, 1], F32)
        nc.vector.memset(epsc[:, :], 1e-5)

        # transposed activations hT as a short ring (written by the LN tail /
        # encoder, read by the T-emit a couple of blocks later) and the
        # SBUF-resident residual stream h [node block, feat]
        hT_sb = const.tile([P, KH, HTR * P], F16)
        h_sb = const.tile([P, NBV, H], F16)

        # z ring (LN-normalized activations, consumed 2 blocks later by PE)
        z_ring = [const.tile([P, H], F16, tag=f"zr{i}", name=f"zr{i}")
                  for i in range(ZR)]

        # zero ALL G pool buffers once so padding rows are finite
        # (uninitialized SBUF can hold NaN bit patterns; 0 * NaN = NaN)
        for b in range(GPA_BUFS):
            ga = gpa.tile([P, SAMAX, H], GDTA, tag="GA")
            nc.vector.memset(ga[:, :, :], 0.0)
        for b in range(GPB_BUFS):
            gb = gpb.tile([P, SBMAX, H], F16, tag="GB")
            nc.vector.memset(gb[:, :, :], 0.0)

        # weights
        encW_t = []
        for k in range(KI):
            row = []
            for m in range(KH):
                t = const.tile([P, P], F16, tag=f"encW{k}{m}")
                nc.sync.dma_start(t[:, :], encW_d[k * P:(k + 1) * P, m * P:(m + 1) * P])
                row.append(t)
            encW_t.append(row)
        encB_c = []
        for m in range(KH):
            t = const.tile([P, 1], F32, tag=f"encB{m}")
            nc.sync.dma_start(t[:, :], encB_d[m * P:(m + 1) * P, None])
            encB_c.append(t)
        thW_t = []
        for i in range(L):
            row = []
            for k in range(KH):
                t = const.tile([P, H], F16, tag=f"thW{i}{k}")
                nc.sync.dma_start(t[:, :], thW_d[i, k * P:(k + 1) * P, :])
                row.append(t)
            thW_t.append(row)
        thB_t = []
        for i in range(L):
            t = const.tile([1, H], F16, tag=f"thB{i}")
            nc.sync.dma_start(t[:, :], thB_d[i:i + 1, :])
            thB_t.append(t)
        linW_t = []
        for k in range(KH):
            t = const.tile([P, OUT], F16, tag=f"linW{k}")
            nc.sync.dma_start(t[:, :], linW_d[k * P:(k + 1) * P, :])
            linW_t.append(t)
        linB_t = const.tile([1, OUT], F16)
        nc.sync.dma_start(linB_t[:, :], linB_d[None, :])
        # LN affine in transposed space: per-feature -> per-partition columns
        lnG_t, lnB_t = [], []
        for i in range(L):
            g = const.tile([P, KH], F32, tag=f"lnG{i}")
            b = const.tile([P, KH], F32, tag=f"lnB{i}")
            nc.sync.dma_start(g[:, :], lnG_d[:, i * KH:(i + 1) * KH])
            nc.sync.dma_start(b[:, :], lnB_d[:, i * KH:(i + 1) * KH])
            lnG_t.append(g)
            lnB_t.append(b)

        # metadata
        idxA_t = meta.tile([P, p.TA16 // 16], I16)
        nc.sync.dma_start(idxA_t[:, :], idxA_d[:, :])
        rA_t = meta.tile([P, p.SLA], F16)
        nc.sync.dma_start(rA_t[:, :], rA_d[:, :])
        dec_t = meta.tile([P, p.NBE], F32)
        nc.sync.dma_start(dec_t[:, :], dec_d[:, :])
        idxB_t = meta.tile([P, p.TB16 // 16], I16)
        nc.sync.dma_start(idxB_t[:, :], idxB_d[:, :])
        rB_t = meta.tile([P, p.SLB], F16)
        nc.sync.dma_start(rB_t[:, :], rB_d[:, :])
        dv_t = meta.tile([P, NBV], F32)
        nc.sync.dma_start(dv_t[:, :], dv_d[:, :])

        CW = 512
        qn_state = [0]

        # r16A/r16B are uniform after host-side balancing: hoist the
        # num_idxs_reg loads so each gather doesn't emit its own MOVE on the
        # Pool stream (dispatch overhead per call)
        uniA = len(set(p.r16A.tolist())) == 1
        uniB = len(set(p.r16B.tolist())) == 1
        regA = nc.gpsimd.to_reg(int(p.r16A[0])) if uniA else None
        regB = nc.gpsimd.to_reg(int(p.r16B[0])) if uniB else None


        def next_q():
            q = qn_state[0]
            qn_state[0] = (q + 1) % 4
            return q

        def emit_T(li, rb):
            # T[rb] = h @ thetaW[li] + thetaB[li], written fp16 to T_d
            hc = (rb % HTR) * P
            psw = psE.tile([P, CW], F32, tag="psE")
            for k in range(KH):
                nc.tensor.matmul(psw[:, :H], lhsT=hT_sb[:, k, hc:hc + P],
                                 rhs=thW_t[li][k][:, :],
                                 start=(k == 0), stop=False)
            nc.tensor.matmul(psw[:, :H], lhsT=ones1[:1, :], rhs=thB_t[li][:1, :],
                             start=False, stop=True)
            Tb = opool.tile([P, H], GDTA, tag="Tout")
            nc.scalar.activation(Tb[:, :], psw[:, :H], AF.Copy)
            nc.sync.dma_start(T_d[rb * P:rb * P + P, :], Tb[:, :])

        def emit_final(rb):
            # out[rb] = t @ linW + linB (t = relu(LN_0(h)) already in hT)
            hc = (rb % HTR) * P
            psw = psE.tile([P, CW], F32, tag="psE")
            for k in range(KH):
                nc.tensor.matmul(psw[:, :OUT], lhsT=hT_sb[:, k, hc:hc + P],
                                 rhs=linW_t[k][:, :], start=(k == 0), stop=False)
            nc.tensor.matmul(psw[:, :OUT], lhsT=ones1[:1, :], rhs=linB_t[:1, :],
                             start=False, stop=True)
            ob = opool.tile([P, OUT], F32, tag="finout")
            nc.scalar.activation(ob[:, :], psw[:, :OUT], AF.Copy)
            nc.sync.dma_start(out_d[rb * P:rb * P + P, :], ob[:, :])

        # ------------------------------------------------------------------
        # Encoder: hT[:, m, :] = (x @ encW + encB)^T, feature-major directly.
        # Layer 0's T-matmul is fused in per 512-column chunk.
        # ------------------------------------------------------------------
        for c0 in range(0, NPAD, CW):
            ncols = min(CW, NPAD - c0)
            hc = c0 % (HTR * P)
            xc = wrk.tile([P, KI, CW], F16, tag="xc")
            nc.sync.dma_start(
                xc[:, :, :ncols],
                xT_d.ap().rearrange("(k q) n -> q k n", q=P)[:, :, c0:c0 + ncols],
            )
            for m in range(KH):
                ps = psE.tile([P, CW], F32, tag="psE")
                for k in range(KI):
                    nc.tensor.matmul(ps[:, :ncols], lhsT=encW_t[k][m][:, :],
                                     rhs=xc[:, k, :ncols],
                                     start=(k == 0), stop=(k == KI - 1))
                nc.scalar.activation(hT_sb[:, m, hc:hc + ncols], ps[:, :ncols],
                                     AF.Identity, bias=encB_c[m][:, :], scale=1.0)
            for rb in range(c0 // P, (c0 + ncols) // P):
                emit_T(0, rb)

        # ------------------------------------------------------------------
        # Conv layers
        # ------------------------------------------------------------------
        def gatherA(eb):
            r16 = int(p.r16A[eb])
            t0 = int(p.tabOffA[eb])
            sb = int(p.slotsA[eb])
            G = gpa.tile([P, SAMAX, H], GDTA, tag="GA")
            nc.gpsimd.dma_gather(
                out_ap=G[:, :sb, :],
                in_ap=T_d[:, :],
                idxs_ap=idxA_t[:, t0 // 16:(t0 + r16) // 16],
                num_idxs=r16,
                num_idxs_reg=regA if uniA else r16,
                elem_size=H,
                queue_num=next_q(),
            )
            return G

        def buildSA(eb):
            sb = int(p.slotsA[eb])
            s0 = int(p.slotOffA[eb])
            S = spool.tile([P, MS, P], GDTA, tag="SA")
            rb_ap = rA_t[:, s0:s0 + sb].unsqueeze(2).broadcast_to([P, sb, P])
            nc.vector.tensor_tensor(S[:, :sb, :], iota_f[:, :sb, :], rb_ap,
                                    op=ALU.is_equal)
            return S

        def gatherB(vb):
            r16 = int(p.r16B[vb])
            t0 = int(p.tabOffB[vb])
            G = gpb.tile([P, SBMAX, H], F16, tag="GB")
            g0 = 0
            while g0 < r16:
                gn = min(1024, r16 - g0)
                nc.gpsimd.dma_gather(
                    out_ap=G[:, g0 // P:g0 // P + _cdiv(gn, P), :],
                    in_ap=YeF_d[:, :],
                    idxs_ap=idxB_t[:, (t0 + g0) // 16:(t0 + g0 + gn) // 16],
                    num_idxs=gn,
                    num_idxs_reg=(regB if (uniB and gn == int(p.r16B[0])) else gn),
                    elem_size=H,
                    queue_num=next_q(),
                )
                g0 += gn
            return G

        def buildSB(vb):
            sb = int(p.slotsB[vb])
            s0 = int(p.slotOffB[vb])
            S = spool.tile([P, MS, P], F16, tag="SB")
            rb_ap = rB_t[:, s0:s0 + sb].unsqueeze(2).broadcast_to([P, sb, P])
            nc.vector.tensor_tensor(S[:, :sb, :], iota_f[:, :sb, :], rb_ap,
                                    op=ALU.is_equal)
            return S

        for li in range(L):
            # ---- Phase A: partial Ye, chunked AllReduce ----
            Gq = {}
            Sq = {}
            for e in range(min(PREGA, NBE_real)):
                Gq[e] = gatherA(e)
            for e in range(min(PRESA, NBE_real)):
                Sq[e] = buildSA(e)
            for eb in range(NBE_real):
                if eb + PREGA < NBE_real:
                    Gq[eb + PREGA] = gatherA(eb + PREGA)
                if eb + PRESA < NBE_real:
                    Sq[eb + PRESA] = buildSA(eb + PRESA)
                G = Gq.pop(eb)
                S = Sq.pop(eb)
                sb = int(p.slotsA[eb])
                ps = psA.tile([P, H], F32, tag="ps256")
                for s in range(sb):
                    nc.tensor.matmul(ps[:, :], lhsT=S[:, s, :], rhs=G[:, s, :],
                                     start=(s == 0), stop=(s == sb - 1))
                yeb = opool.tile([P, H], F16, tag="yeg", bufs=4)
                ck = next(k for k in range(NCHUNK) if CHB[k] <= eb < CHB[k + 1])
                er = (eb - CHB[ck]) * P
                nc.scalar.activation(yeb[:, :], ps[:, :], AF.Copy,
                                     scale=dec_t[:, eb:eb + 1])
                nc.sync.dma_start(YePc_d[ck][er:er + P, :], yeb[:, :])

                # AllReduce each chunk 12 blocks after its last block so the
                # Pool stream never stalls on the chunk's YeP write sems
                # (a stalled Pool head starves gather dispatch for ~24us)
                if eb >= 12 and (eb - 12 + 1) in CHB[1:NCHUNK]:
                    ck2 = CHB.index(eb - 12 + 1) - 1
                    nc.gpsimd.collective_compute(
                        "AllReduce", ALU.add,
                        replica_groups=[list(range(C))],
                        ins=[YePc_d[ck2].ap()[:, :]],
                        outs=[YeF_d.ap()[CHB[ck2] * P:CHB[ck2 + 1] * P, :]],
                    )
            nc.gpsimd.collective_compute(
                "AllReduce", ALU.add,
                replica_groups=[list(range(C))],
                ins=[YePc_d[NCHUNK - 1].ap()[:, :]],
                outs=[YeF_d.ap()[CHB[NCHUNK - 1] * P:, :]],
            )

            # ---- Phase B: conv + residual + LN tail (PE tail deferred) ----
            lnxt = li + 1 if li + 1 < L else 0

            def tail_transpose(vt):
                z = z_ring[vt % ZR]
                hc = (vt % HTR) * P
                for m in range(KH):
                    pst = psT.tile([P, P], F16, tag="psT")
                    nc.tensor.transpose(pst[:, :], z[:, m * P:(m + 1) * P],
                                        ident[:, :])
                    nc.scalar.activation(
                        hT_sb[:, m, hc:hc + P], pst[:, :], AF.Relu,
                        bias=lnB_t[lnxt][:, m:m + 1], scale=lnG_t[lnxt][:, m:m + 1])

            def emit_at(vt):
                if li + 1 < L:
                    emit_T(li + 1, vt)
                else:
                    emit_final(vt)

            Gq = {}
            Sq = {}
            for v in range(min(PREGB, NBV)):
                Gq[v] = gatherB(v)
            for v in range(min(PRESB, NBV)):
                Sq[v] = buildSB(v)
            for vb in range(NBV):
                if vb + PREGB < NBV:
                    Gq[vb + PREGB] = gatherB(vb + PREGB)
                if vb + PRESB < NBV:
                    Sq[vb + PRESB] = buildSB(vb + PRESB)
                G = Gq.pop(vb)
                S = Sq.pop(vb)
                sb = int(p.slotsB[vb])
                ps = psA.tile([P, H], F32, tag="ps256")
                for s in range(sb):
                    nc.tensor.matmul(ps[:, :], lhsT=S[:, s, :], rhs=G[:, s, :],
                                     start=(s == 0), stop=(s == sb - 1))
                # hn = relu(dv * x) (== dv * relu(x), dv >= 0), on DVE
                hslice = h_sb[:, vb, :]
                if li == 0:
                    nc.vector.tensor_scalar(hslice, ps[:, :], dv_t[:, vb:vb + 1],
                                            0.0, op0=ALU.mult, op1=ALU.max)
                else:
                    hn = wrk.tile([P, H], F16, tag="hn")
                    nc.vector.tensor_scalar(hn[:, :], ps[:, :], dv_t[:, vb:vb + 1],
                                            0.0, op0=ALU.mult, op1=ALU.max)
                    nc.vector.tensor_add(hslice, hslice, hn[:, :])
                # tail: z = (h - mu) * rinv (transpose + affine deferred)
                st6 = stat.tile([P, 6], F32, tag="st6")
                nc.vector.bn_stats(st6[:, :], hslice)
                mv = stat.tile([P, 2], F32, tag="mv")
                nc.vector.bn_aggr(mv[:, :], st6[:, :])
                rstd = stat.tile([P, 1], F32, tag="rstd")
                nc.scalar.activation(rstd[:, :], mv[:, 1:2], AF.Sqrt,
                                     bias=epsc[:, :], scale=1.0)
                rinv = stat.tile([P, 1], F32, tag="rinv")
                nc.vector.reciprocal(rinv[:, :], rstd[:, :])
                nmr = stat.tile([P, 1], F32, tag="nmr")
                nc.vector.tensor_scalar(nmr[:, :], mv[:, 0:1], rinv[:, :], -1.0,
                                        op0=ALU.mult, op1=ALU.mult)
                z = z_ring[vb % ZR]
                nc.scalar.activation(z[:, :], hslice, AF.Identity,
                                     bias=nmr[:, :], scale=rinv[:, :])
                if vb >= 2:
                    tail_transpose(vb - 2)
                if vb >= 4:
                    emit_at(vb - 4)
            tail_transpose(NBV - 2)
            tail_transpose(NBV - 1)
            for vt in (NBV - 4, NBV - 3, NBV - 2, NBV - 1):
                emit_at(vt)

    nc.compile()
    return nc


# ----------------------------------------------------------------------------
# Full pipeline: prep + build + run
# ----------------------------------------------------------------------------
def run_full(x, vidx, eidx, encW, encB, thetaW, thetaB, lnG, lnB, linW, linB,
             N, E, C, trace=False, nc_cache=None, **runkw):
    IN_DIM = x.shape[1]
    H = encW.shape[1]
    OUT = linW.shape[1]
    L = thetaW.shape[0]

    p = host_prep(np.asarray(vidx), np.asarray(eidx), N, E, C)
    nc = nc_cache if nc_cache is not None else build_program(p, IN_DIM, H, OUT, L)

    x = np.asarray(x, np.float32)
    NP, NPAD = p.NP, p.NPAD
    in_maps = []
    for c in range(C):
        xs = x[c * NP:(c + 1) * NP]
        xT = np.zeros((IN_DIM, NPAD), F16_NP)
        xT[:, p.slot_of[c]] = xs.T.astype(F16_NP)
        in_maps.append(dict(
            xT=xT,
            encW=np.asarray(encW, F16_NP),
            encB=np.asarray(encB, np.float32),
            thW=np.asarray(thetaW, F16_NP),
            thB=np.asarray(thetaB, F16_NP),
            lnGT=np.ascontiguousarray(
                np.asarray(lnG, np.float32).reshape(4, 2, P).transpose(2, 0, 1).reshape(P, 8)),
            lnBT=np.ascontiguousarray(
                np.asarray(lnB, np.float32).reshape(4, 2, P).transpose(2, 0, 1).reshape(P, 8)),
            linW=np.asarray(linW, F16_NP),
            linB=np.asarray(linB, F16_NP),
            idxA=p.idxA_w[c],
            rA=p.rA_m[c],
            idxB=p.idxB_w[c],
            rB=p.rB_m[c],
            dvc=p.dvc[c],
            dec=p.dec,
        ))

    res = run_bass_kernel_spmd(nc, in_maps, core_ids=list(range(C)), trace=trace, **runkw)
    out = np.concatenate(
        [res.results[c]["out"][p.slot_of[c]] for c in range(C)], axis=0)
    return out, res, nc, p


# hardcoded problem configuration (nn_DeeperHNN_88295937671288)
_N, _E, _NNZ = 100000, 20000, 800000
_C = 8

_nc_cache = None


def kernel(x, vidx, eidx, encW, encB, thetaW, thetaB, lnG, lnB, linW, linB):
    global _nc_cache
    out, res, nc, p = run_full(
        x, vidx, eidx, encW, encB, thetaW, thetaB, lnG, lnB, linW, linB,
        N=_N, E=_E, C=_C, nc_cache=None,
    )
    _nc_cache = nc
    return out.astype(np.float32)
